# revision 3
# baseline (speedup 1.0000x reference)
"""Trainium2 Bass kernel for a relational GCN layer (message passing + LSTM + MLP).

Math (reference):
  S = feat[src]                               # [E, D] gather
  msgs[e] = edge_nn(S[e], W_rel[rel[e]])      # tied 2-layer relu MLP per relation
  agg = segment_sum(msgs, dst, N)             # [N, D]
  hn = LSTM-step(agg) from zero state         # [N, D]
  out = MLP(hn)                               # [N, D_OUT]

Key algebraic optimization: msgs[e] depends only on (rel[e], src[e]) of which
there are NUM_REL*N = 20k combos << E = 320k.  So precompute the transformed
node table H[r*N + s] = edge_nn(feat[s], W_rel[r]) once (dense GEMMs), then
the whole edge phase collapses to gather-rows + segment-sum.

Distribution: edges are sharded by DESTINATION node range across the 8 cores
(core c owns dst in [1250c, 1250(c+1))), so every core computes complete
aggregates for its own 1250 nodes and NO cross-core reduction is needed.
Each core computes the full H table (duplicated compute, zero communication).

Per core:
  phase A: H table build    — dense GEMMs, H stored to local HBM
  phase B: per 128-dst-node block: indirect-DMA gather of H rows per 128-edge
           tile + one-hot selector matmul accumulating into PSUM (segment sum)
  phase C: LSTM + MLP on the 128-node block, output written transposed
Host: index preprocessing (sort edges by dst, build per-tile gather indices
and in-block dst offsets) and final unshard/transpose.
"""

import math
import numpy as np
import ml_dtypes

import concourse.bacc as bacc
import concourse.bass as bass
import concourse.mybir as mybir
import concourse.tile as tile
from concourse import bass_utils
from concourse.masks import make_identity

# ---- problem constants (hardcoded per spec) ----
N_NODES = 10000
N_EDGES = 320000
D = 256
D_OUT = 256
NUM_REL = 2
NCORES = 8
NPC = N_NODES // NCORES          # 1250 nodes per core
NBLK = math.ceil(NPC / 128)      # 10 dst blocks per core (last has 98 rows)
NPAD = 10240                     # node count padded to 20 tiles of 512
NT_A = NPAD // 512               # 20 node tiles in phase A

f32 = mybir.dt.float32
bf16 = mybir.dt.bfloat16
i32 = mybir.dt.int32

# dtype config: (phase-A matmul dt, H table dt, phase-C matmul dt)
DT_CFG = {
    "bf16": (bf16, bf16, bf16),
    "f32": (f32, f32, f32),
}
_np_dt = {bf16: ml_dtypes.bfloat16, f32: np.float32}

PAD_DLOC = 999.0  # sentinel: one-hot selector row all-zero -> edge ignored


# ----------------------------------------------------------------------------
# host-side preprocessing
# ----------------------------------------------------------------------------

def _prep_edges(src, dst, rel):
    """Sort edges by dst; shard by dst range; build per-tile gather index and
    in-block dst-offset arrays with a common static tiling across cores.

    Returns (tiles_per_block [NBLK], gidx_all [NCORES,128,NT], dloc_all [...]).
    """
    order = np.argsort(dst, kind="stable")
    dst_s = dst[order]
    gidx_s = (rel[order].astype(np.int64) * NPAD + src[order]).astype(np.int32)

    # per (core, block) edge counts
    core_s = dst_s // NPC
    blk_s = (dst_s % NPC) // 128
    dloc_s = ((dst_s % NPC) % 128).astype(np.float32)

    counts = np.zeros((NCORES, NBLK), dtype=np.int64)
    np.add.at(counts, (core_s, blk_s), 1)
    tiles_pb = [int(math.ceil(counts[:, b].max() / 128)) for b in range(NBLK)]
    nt = sum(tiles_pb)
    tile_ofs = np.cumsum([0] + tiles_pb)

    gidx_all = np.zeros((NCORES, 128, nt), dtype=np.int32)
    dloc_all = np.full((NCORES, 128, nt), PAD_DLOC, dtype=np.float32)

    # edges are sorted by dst => grouped by (core, block) contiguously
    starts = np.zeros((NCORES, NBLK), dtype=np.int64)
    flat_counts = counts.reshape(-1)
    flat_starts = np.concatenate([[0], np.cumsum(flat_counts)[:-1]])
    starts[:] = flat_starts.reshape(NCORES, NBLK)

    for c in range(NCORES):
        for b in range(NBLK):
            n = int(counts[c, b])
            if n == 0:
                continue
            s = int(starts[c, b])
            g = gidx_s[s:s + n]
            d = dloc_s[s:s + n]
            k = np.arange(n)
            cols = tile_ofs[b] + k // 128
            parts = k % 128
            gidx_all[c, parts, cols] = g
            dloc_all[c, parts, cols] = d
    return tiles_pb, gidx_all, dloc_all


def _prep_weights(inputs, mm_a_np, mm_c_np):
    feat = np.asarray(inputs["feat"], dtype=np.float32)
    W_rel = np.asarray(inputs["W_rel"], dtype=np.float32)
    b_rel = np.asarray(inputs["b_rel"], dtype=np.float32)
    W_ih = np.asarray(inputs["W_ih"], dtype=np.float32)
    b_ih = np.asarray(inputs["b_ih"], dtype=np.float32)
    b_hh = np.asarray(inputs["b_hh"], dtype=np.float32)
    W1 = np.asarray(inputs["W1"], dtype=np.float32)
    b1 = np.asarray(inputs["b1"], dtype=np.float32)
    W2 = np.asarray(inputs["W2"], dtype=np.float32)
    b2 = np.asarray(inputs["b2"], dtype=np.float32)
    W3 = np.asarray(inputs["W3"], dtype=np.float32)
    b3 = np.asarray(inputs["b3"], dtype=np.float32)

    featT = np.zeros((D, NPAD), dtype=np.float32)
    featT[:, :N_NODES] = feat.T
    keep = np.r_[0:256, 512:1024]  # i, g, o gate columns (f unused: c0 = 0)
    com = {
        "featT": featT.astype(mm_a_np),
        "W_rT": np.ascontiguousarray(np.transpose(W_rel, (0, 2, 1))).astype(mm_a_np),
        "b_r_col": np.ascontiguousarray(b_rel[:, :, None]),                 # f32
        "b_r_row": np.ascontiguousarray(b_rel[:, None, :]).astype(mm_a_np),
        "W_ihT": np.ascontiguousarray(W_ih.T[:, keep]).astype(mm_c_np),     # [256,768]
        "b_g_row": np.ascontiguousarray((b_ih + b_hh)[keep][None, :]).astype(mm_c_np),
        "W1T": np.ascontiguousarray(W1.T).astype(mm_c_np),                  # [256,128]
        "b1_col": np.ascontiguousarray(b1[:, None]),
        "W2T": np.ascontiguousarray(W2.T).astype(mm_c_np),                  # [128,128]
        "b2_col": np.ascontiguousarray(b2[:, None]),
        "W3T": np.ascontiguousarray(W3.T).astype(mm_c_np),                  # [128,256]
        "b3_col": np.ascontiguousarray(b3.reshape(2, 128, 1)),
        "iota": np.tile(np.arange(128, dtype=np.float32), (128, 1)),
    }
    return com


# ----------------------------------------------------------------------------
# kernel builder
# ----------------------------------------------------------------------------

def _build(tiles_pb, dtcfg):
    mm_a, h_dt, mm_c = DT_CFG[dtcfg]
    nt = sum(tiles_pb)
    Relu = mybir.ActivationFunctionType.Relu
    Sig = mybir.ActivationFunctionType.Sigmoid
    Tanh = mybir.ActivationFunctionType.Tanh
    Copy = mybir.ActivationFunctionType.Copy

    nc = bacc.Bacc("TRN2", target_bir_lowering=False, debug=False)

    featT_d = nc.dram_tensor("featT", [D, NPAD], mm_a, kind="ExternalInput")
    W_rT_d = nc.dram_tensor("W_rT", [NUM_REL, D, D], mm_a, kind="ExternalInput")
    b_r_col_d = nc.dram_tensor("b_r_col", [NUM_REL, D, 1], f32, kind="ExternalInput")
    b_r_row_d = nc.dram_tensor("b_r_row", [NUM_REL, 1, D], mm_a, kind="ExternalInput")
    W_ihT_d = nc.dram_tensor("W_ihT", [D, 768], mm_c, kind="ExternalInput")
    b_g_row_d = nc.dram_tensor("b_g_row", [1, 768], mm_c, kind="ExternalInput")
    W1T_d = nc.dram_tensor("W1T", [D, 128], mm_c, kind="ExternalInput")
    b1_col_d = nc.dram_tensor("b1_col", [128, 1], f32, kind="ExternalInput")
    W2T_d = nc.dram_tensor("W2T", [128, 128], mm_c, kind="ExternalInput")
    b2_col_d = nc.dram_tensor("b2_col", [128, 1], f32, kind="ExternalInput")
    W3T_d = nc.dram_tensor("W3T", [128, D_OUT], mm_c, kind="ExternalInput")
    b3_col_d = nc.dram_tensor("b3_col", [2, 128, 1], f32, kind="ExternalInput")
    iota_d = nc.dram_tensor("iota", [128, 128], f32, kind="ExternalInput")
    gidx_d = nc.dram_tensor("gidx", [128, nt], i32, kind="ExternalInput")
    dloc_d = nc.dram_tensor("dloc", [128, nt], f32, kind="ExternalInput")

    outT_d = nc.dram_tensor("outT", [D_OUT, NPC], f32, kind="ExternalOutput")

    H_d = nc.dram_tensor("Htab", [NUM_REL * NPAD, D], h_dt)  # internal HBM

    with tile.TileContext(nc) as tc:
        with (
            tc.tile_pool(name="const", bufs=1) as cp,
            tc.tile_pool(name="work", bufs=3) as wp,
        ):
            # ---- load constants to SBUF ----
            W_rT_sb = {}
            for r in range(NUM_REL):
                for h in range(2):
                    t = cp.tile([128, D], mm_a, tag=f"wrt{r}{h}")
                    nc.sync.dma_start(t[:], W_rT_d[r, h * 128:(h + 1) * 128, :])
                    W_rT_sb[r, h] = t
            b_r_col_sb = {}
            for r in range(NUM_REL):
                for h in range(2):
                    t = cp.tile([128, 1], f32, tag=f"brc{r}{h}")
                    nc.sync.dma_start(t[:], b_r_col_d[r, h * 128:(h + 1) * 128, :])
                    b_r_col_sb[r, h] = t
            b_r_row_sb = {}
            for r in range(NUM_REL):
                t = cp.tile([1, D], mm_a, tag=f"brr{r}")
                nc.sync.dma_start(t[:], b_r_row_d[r, :, :])
                b_r_row_sb[r] = t
            W_ihT_sb = {}
            for h in range(2):
                t = cp.tile([128, 768], mm_c, tag=f"wih{h}")
                nc.sync.dma_start(t[:], W_ihT_d[h * 128:(h + 1) * 128, :])
                W_ihT_sb[h] = t
            b_g_row_sb = cp.tile([1, 768], mm_c, tag="bg")
            nc.sync.dma_start(b_g_row_sb[:], b_g_row_d[:, :])
            W1T_sb = {}
            for h in range(2):
                t = cp.tile([128, 128], mm_c, tag=f"w1t{h}")
                nc.sync.dma_start(t[:], W1T_d[h * 128:(h + 1) * 128, :])
                W1T_sb[h] = t
            b1_col_sb = cp.tile([128, 1], f32, tag="b1")
            nc.sync.dma_start(b1_col_sb[:], b1_col_d[:, :])
            W2T_sb = cp.tile([128, 128], mm_c, tag="w2t")
            nc.sync.dma_start(W2T_sb[:], W2T_d[:, :])
            b2_col_sb = cp.tile([128, 1], f32, tag="b2")
            nc.sync.dma_start(b2_col_sb[:], b2_col_d[:, :])
            W3T_sb = cp.tile([128, D_OUT], mm_c, tag="w3t")
            nc.sync.dma_start(W3T_sb[:], W3T_d[:, :])
            b3_col_sb = {}
            for h in range(2):
                t = cp.tile([128, 1], f32, tag=f"b3{h}")
                nc.sync.dma_start(t[:], b3_col_d[h, :, :])
                b3_col_sb[h] = t
            iota_sb = cp.tile([128, 128], f32, tag="iota")
            nc.sync.dma_start(iota_sb[:], iota_d[:, :])
            gidx_sb = cp.tile([128, nt], i32, tag="gidx")
            nc.sync.dma_start(gidx_sb[:], gidx_d[:, :])
            dloc_sb = cp.tile([128, nt], f32, tag="dloc")
            nc.sync.dma_start(dloc_sb[:], dloc_d[:, :])
            ident = cp.tile([128, 128], f32, tag="ident")
            make_identity(nc, ident[:])
            ones_a = cp.tile([1, 128], mm_a, tag="onesa")
            nc.gpsimd.memset(ones_a[:], 1.0)
            ones_c = cp.tile([1, 128], mm_c, tag="onesc")
            nc.gpsimd.memset(ones_c[:], 1.0)

            # ---- phase A: build H table ----
            with tc.tile_pool(name="psA", bufs=1, space="PSUM") as psA:
                for r in range(NUM_REL):
                    for ntl in range(NT_A):
                        c0 = ntl * 512
                        ft = {}
                        for h in range(2):
                            t = wp.tile([128, 512], mm_a, tag=f"ft{h}")
                            nc.sync.dma_start(
                                t[:], featT_d[h * 128:(h + 1) * 128, c0:c0 + 512])
                            ft[h] = t
                        z1s = {}
                        for do_h in range(2):
                            z1p = psA.tile([128, 512], f32, tag=f"z1_{do_h}",
                                           space="PSUM")
                            for di_h in range(2):
                                nc.tensor.matmul(
                                    z1p[:],
                                    lhsT=W_rT_sb[r, di_h][:, do_h * 128:(do_h + 1) * 128],
                                    rhs=ft[di_h][:],
                                    start=(di_h == 0), stop=(di_h == 1))
                            z = wp.tile([128, 512], mm_a, tag=f"z1s{do_h}")
                            nc.scalar.activation(z[:], z1p[:], Relu,
                                                 bias=b_r_col_sb[r, do_h][:], scale=1.0)
                            z1s[do_h] = z
                        for c4 in range(4):
                            hp = psA.tile([128, D], f32, tag="hp", space="PSUM")
                            sl = slice(c4 * 128, (c4 + 1) * 128)
                            nc.tensor.matmul(hp[:], lhsT=z1s[0][:, sl], rhs=W_rT_sb[r, 0][:],
                                             start=True, stop=False)
                            nc.tensor.matmul(hp[:], lhsT=z1s[1][:, sl], rhs=W_rT_sb[r, 1][:],
                                             start=False, stop=False)
                            nc.tensor.matmul(hp[:], lhsT=ones_a[:], rhs=b_r_row_sb[r][:],
                                             start=False, stop=True)
                            hs = wp.tile([128, D], h_dt, tag="hs")
                            nc.scalar.activation(hs[:], hp[:], Relu, bias=0.0, scale=1.0)
                            row0 = r * NPAD + c0 + c4 * 128
                            nc.sync.dma_start(H_d[row0:row0 + 128, :], hs[:])

            # ---- phase B + C per dst block ----
            tile_ofs = np.cumsum([0] + list(tiles_pb))
            with tc.tile_pool(name="psB", bufs=1, space="PSUM") as psB:
                for b in range(NBLK):
                    tb = tiles_pb[b]
                    aggp = psB.tile([128, D], f32, tag="agg", space="PSUM", bufs=2)
                    for t in range(tb):
                        j = int(tile_ofs[b]) + t
                        m = wp.tile([128, D], h_dt, tag="m", bufs=6)
                        nc.gpsimd.indirect_dma_start(
                            out=m[:], out_offset=None,
                            in_=H_d[:],
                            in_offset=bass.IndirectOffsetOnAxis(
                                ap=gidx_sb[:, j:j + 1], axis=0),
                        )
                        sel = wp.tile([128, 128], h_dt, tag="sel", bufs=6)
                        nc.vector.tensor_tensor(
                            out=sel[:],
                            in0=dloc_sb[:, j:j + 1].to_broadcast([128, 128]),
                            in1=iota_sb[:], op=mybir.AluOpType.is_equal)
                        nc.tensor.matmul(aggp[:], lhsT=sel[:], rhs=m[:],
                                         start=(t == 0), stop=(t == tb - 1))

                    # --- phase C ---
                    nn = min(128, NPC - b * 128)
                    agg_sb = wp.tile([128, D], f32, tag="aggsb")
                    nc.vector.tensor_copy(agg_sb[:], aggp[:])
                    aggT = {}
                    for h in range(2):
                        tp = psB.tile([128, 128], f32, tag="tp", space="PSUM", bufs=2)
                        nc.tensor.transpose(tp[:], agg_sb[:, h * 128:(h + 1) * 128],
                                            ident[:])
                        at = wp.tile([128, 128], mm_c, tag=f"aggT{h}")
                        nc.vector.tensor_copy(at[:], tp[:])
                        aggT[h] = at
                    # gates: bank0 = i (256 cols), bank1 = g|o (512 cols)
                    g0 = psB.tile([128, 256], f32, tag="g0", space="PSUM")
                    g1 = psB.tile([128, 512], f32, tag="g1", space="PSUM")
                    for h in range(2):
                        nc.tensor.matmul(g0[:], lhsT=aggT[h][:], rhs=W_ihT_sb[h][:, 0:256],
                                         start=(h == 0), stop=False)
                    nc.tensor.matmul(g0[:], lhsT=ones_c[:], rhs=b_g_row_sb[:, 0:256],
                                     start=False, stop=True)
                    for h in range(2):
                        nc.tensor.matmul(g1[:], lhsT=aggT[h][:], rhs=W_ihT_sb[h][:, 256:768],
                                         start=(h == 0), stop=False)
                    nc.tensor.matmul(g1[:], lhsT=ones_c[:], rhs=b_g_row_sb[:, 256:768],
                                     start=False, stop=True)
                    si = wp.tile([128, 256], f32, tag="si")
                    nc.scalar.activation(si[:], g0[:], Sig, bias=0.0, scale=1.0)
                    tg = wp.tile([128, 256], f32, tag="tg")
                    nc.scalar.activation(tg[:], g1[:, 0:256], Tanh, bias=0.0, scale=1.0)
                    so = wp.tile([128, 256], f32, tag="so")
                    nc.scalar.activation(so[:], g1[:, 256:512], Sig, bias=0.0, scale=1.0)
                    cc = wp.tile([128, 256], f32, tag="cc")
                    nc.vector.tensor_mul(cc[:], si[:], tg[:])
                    tcc = wp.tile([128, 256], f32, tag="tcc")
                    nc.scalar.activation(tcc[:], cc[:], Tanh, bias=0.0, scale=1.0)
                    hn = wp.tile([128, 256], f32, tag="hn")
                    nc.vector.tensor_mul(hn[:], so[:], tcc[:])
                    hnT = {}
                    for h in range(2):
                        tp = psB.tile([128, 128], f32, tag="tp", space="PSUM", bufs=2)
                        nc.tensor.transpose(tp[:], hn[:, h * 128:(h + 1) * 128], ident[:])
                        ht = wp.tile([128, 128], mm_c, tag=f"hnT{h}")
                        nc.vector.tensor_copy(ht[:], tp[:])
                        hnT[h] = ht
                    # MLP (all in transposed activation layout: [feature, node])
                    x12 = psB.tile([128, 256], f32, tag="x12", space="PSUM")
                    for h in range(2):
                        nc.tensor.matmul(x12[:, 0:128], lhsT=W1T_sb[h][:], rhs=hnT[h][:],
                                         start=(h == 0), stop=(h == 1))
                    x1s = wp.tile([128, 128], mm_c, tag="x1s")
                    nc.scalar.activation(x1s[:], x12[:, 0:128], Relu,
                                         bias=b1_col_sb[:], scale=1.0)
                    nc.tensor.matmul(x12[:, 128:256], lhsT=W2T_sb[:], rhs=x1s[:],
                                     start=True, stop=True)
                    x2s = wp.tile([128, 128], mm_c, tag="x2s")
                    nc.scalar.activation(x2s[:], x12[:, 128:256], Relu,
                                         bias=b2_col_sb[:], scale=1.0)
                    outp = psB.tile([128, 256], f32, tag="outp", space="PSUM")
                    for oh in range(2):
                        nc.tensor.matmul(outp[:, oh * 128:(oh + 1) * 128],
                                         lhsT=W3T_sb[:, oh * 128:(oh + 1) * 128],
                                         rhs=x2s[:], start=True, stop=True)
                    for oh in range(2):
                        osb = wp.tile([128, 128], f32, tag=f"osb{oh}")
                        nc.vector.tensor_scalar_add(
                            osb[:], outp[:, oh * 128:(oh + 1) * 128], b3_col_sb[oh][:])
                        nc.sync.dma_start(
                            outT_d[oh * 128:(oh + 1) * 128, b * 128:b * 128 + nn],
                            osb[:, 0:nn])

    nc.compile()
    return nc


_CACHE = {}


def _get_nc(tiles_key, dtcfg):
    key = (tiles_key, dtcfg)
    if key not in _CACHE:
        _CACHE[key] = _build(list(tiles_key), dtcfg)
    return _CACHE[key]


# ----------------------------------------------------------------------------
# public entry
# ----------------------------------------------------------------------------

def kernel(dtcfg="bf16", **inputs) -> np.ndarray:
    src = np.asarray(inputs["src"], dtype=np.int32)
    dst = np.asarray(inputs["dst"], dtype=np.int32)
    rel = np.asarray(inputs["rel"], dtype=np.int32)

    mm_a, h_dt, mm_c = DT_CFG[dtcfg]
    com = _prep_weights(inputs, _np_dt[mm_a], _np_dt[mm_c])
    tiles_pb, gidx_all, dloc_all = _prep_edges(src, dst, rel)

    nc = _get_nc(tuple(tiles_pb), dtcfg)

    in_maps = []
    for c in range(NCORES):
        m = dict(com)
        m["gidx"] = np.ascontiguousarray(gidx_all[c])
        m["dloc"] = np.ascontiguousarray(dloc_all[c])
        in_maps.append(m)

    res = bass_utils.run_bass_kernel_spmd(nc, in_maps, core_ids=list(range(NCORES)))

    out = np.empty((N_NODES, D_OUT), dtype=np.float32)
    for c in range(NCORES):
        out[c * NPC:(c + 1) * NPC, :] = res.results[c]["outT"].T
    return out


# revision 15
# speedup vs baseline: 1.3645x; 1.3645x over previous
"""Trainium2 Bass kernel for a relational GCN layer (message passing + LSTM + MLP).

Math (reference):
  S = feat[src]                               # [E, D] gather
  msgs[e] = edge_nn(S[e], W_rel[rel[e]])      # tied 2-layer relu MLP per relation
  agg = segment_sum(msgs, dst, N)             # [N, D]
  hn = LSTM-step(agg) from zero state         # [N, D]
  out = MLP(hn)                               # [N, D_OUT]

Key algebraic optimization: msgs[e] depends only on (rel[e], src[e]) of which
there are NUM_REL*N = 20k combos << E = 320k.  So precompute the transformed
node table H[r, s] = edge_nn(feat[s], W_rel[r]) once (dense GEMMs), then the
whole edge phase collapses to row-gather + segment-sum.

Distribution: edges are sharded by DESTINATION node range across the 8 cores
(core c owns dst in [1250c, 1250(c+1))), so every core computes complete
aggregates for its own 1250 nodes and no cross-core communication is needed
(on-chip collectives are ~60 GB/s — slower than recomputing H per core).

Per core:
  phase A: H table build (dense GEMMs), split into lo/hi src halves written to
           separate HBM tensors so phase-B gathers for the lo half can start
           while the hi half is still computing (PE work overlaps the
           GPSIMD-serialized gather descriptor generation).
  phase B: per 128-dst-node block: one batched dma_gather per (block, half)
           pulls the edge messages; a one-hot selector (built on DVE from the
           in-block dst offset via is_equal against an iota row) feeds
           matmuls that accumulate the segment sum in PSUM.
  phase C: LSTM + MLP on the 128-node block (activations kept transposed so
           weights stay stationary and biases land on partitions), output
           written transposed; host reassembles.
"""

import math
import numpy as np
import ml_dtypes

import concourse.bacc as bacc
import concourse.bass as bass
import concourse.mybir as mybir
import concourse.tile as tile
from concourse import bass_utils
from concourse.masks import make_identity
from concourse.tile import add_dep_helper

# ---- problem constants (hardcoded per spec) ----
N_NODES = 10000
N_EDGES = 320000
D = 256
D_OUT = 256
NUM_REL = 2
NCORES = 8
NPC = N_NODES // NCORES          # 1250 nodes per core
NBLK = math.ceil(NPC / 128)      # 10 dst blocks per core (last has 98 rows)
NPAD = 10240                     # node count padded to 20 tiles of 512
NH = NPAD // 2                   # 5120 nodes per src half
NT_H = NH // 512                 # 10 node tiles per half (per relation)

f32 = mybir.dt.float32
bf16 = mybir.dt.bfloat16
i16 = mybir.dt.int16

# dtype config: (phase-A matmul dt, H table dt, phase-C matmul dt)
DT_CFG = {
    "bf16": (bf16, bf16, bf16),
    "f32": (f32, f32, f32),
}
_np_dt = {bf16: ml_dtypes.bfloat16, f32: np.float32}

PAD_DLOC = 999.0  # sentinel: one-hot selector row all-zero -> edge ignored


# ----------------------------------------------------------------------------
# host-side preprocessing
# ----------------------------------------------------------------------------

def _prep_edges(src, dst, rel):
    """Sort edges by (dst, src-half); shard by dst range; build the dma_gather
    index array and in-block dst-offset array with a static tiling common to
    all cores.

    Tiles are grouped per (dst-block, src-half): block b has tiles_pb[b][0]
    tiles whose edges have src < NH (gathered from Htab0) followed by
    tiles_pb[b][1] tiles with src >= NH (gathered from Htab1).

    Edge i of a group lands at msup[i % 128, i // 128, :] (dma_gather layout),
    so its gather index goes to idxs16[i % 16 (+16g), col*8 + i // 16] and its
    dst offset to dloc[i % 128, col + i // 128].
    """
    half = (src >= NH).astype(np.int64)
    core = dst // NPC
    blk = (dst % NPC) // 128
    group_key = (core * NBLK + blk) * 2 + half
    order = np.argsort(group_key, kind="stable")
    dst_s = dst[order]
    half_s = half[order]
    row_s = (rel[order].astype(np.int64) * NH + (src[order] % NH)).astype(np.int16)

    core_s = dst_s // NPC
    blk_s = (dst_s % NPC) // 128
    dloc_s = ((dst_s % NPC) % 128).astype(np.float32)

    counts = np.zeros((NCORES, NBLK, 2), dtype=np.int64)
    np.add.at(counts, (core_s, blk_s, half_s), 1)
    tiles_pb = [[int(math.ceil(counts[:, b, g].max() / 128)) for g in range(2)]
                for b in range(NBLK)]
    nt = sum(t for pair in tiles_pb for t in pair)
    grp_ofs = {}
    acc = 0
    for b in range(NBLK):
        for g in range(2):
            grp_ofs[b, g] = acc
            acc += tiles_pb[b][g]

    idxs16_all = np.zeros((NCORES, 16, nt * 8), dtype=np.int16)
    dloc_all = np.full((NCORES, 128, nt), PAD_DLOC, dtype=np.float32)

    flat_counts = counts.reshape(-1)
    flat_starts = np.concatenate([[0], np.cumsum(flat_counts)[:-1]])
    starts = flat_starts.reshape(NCORES, NBLK, 2)

    for c in range(NCORES):
        for b in range(NBLK):
            for g in range(2):
                n = int(counts[c, b, g])
                if n == 0:
                    continue
                s = int(starts[c, b, g])
                k = np.arange(n)
                ofs = grp_ofs[b, g]
                idxs16_all[c, k % 16, ofs * 8 + k // 16] = row_s[s:s + n]
                dloc_all[c, k % 128, ofs + k // 128] = dloc_s[s:s + n]
    idxs16_all = np.tile(idxs16_all, (1, 8, 1))
    return tiles_pb, idxs16_all, dloc_all


def _prep_weights(inputs, mm_a_np, mm_c_np):
    feat = np.asarray(inputs["feat"], dtype=np.float32)
    W_rel = np.asarray(inputs["W_rel"], dtype=np.float32)
    b_rel = np.asarray(inputs["b_rel"], dtype=np.float32)
    W_ih = np.asarray(inputs["W_ih"], dtype=np.float32)
    b_ih = np.asarray(inputs["b_ih"], dtype=np.float32)
    b_hh = np.asarray(inputs["b_hh"], dtype=np.float32)
    W1 = np.asarray(inputs["W1"], dtype=np.float32)
    b1 = np.asarray(inputs["b1"], dtype=np.float32)
    W2 = np.asarray(inputs["W2"], dtype=np.float32)
    b2 = np.asarray(inputs["b2"], dtype=np.float32)
    W3 = np.asarray(inputs["W3"], dtype=np.float32)
    b3 = np.asarray(inputs["b3"], dtype=np.float32)

    featT = np.zeros((D, NPAD), dtype=np.float32)
    featT[:, :N_NODES] = feat.T
    keep = np.r_[0:256, 512:1024]  # i, g, o gate columns (f unused: c0 = 0)
    com = {
        "featT": featT.astype(mm_a_np),
        "W_rT": np.ascontiguousarray(np.transpose(W_rel, (0, 2, 1))).astype(mm_a_np),
        "b_r_col": np.ascontiguousarray(b_rel[:, :, None]),                 # f32
        "b_r_row": np.ascontiguousarray(b_rel[:, None, :]).astype(mm_a_np),
        "W_ihT": np.ascontiguousarray(W_ih.T[:, keep]).astype(mm_c_np),     # [256,768]
        "b_g_row": np.ascontiguousarray((b_ih + b_hh)[keep][None, :]).astype(mm_c_np),
        "W1T": np.ascontiguousarray(W1.T).astype(mm_c_np),                  # [256,128]
        "b1_col": np.ascontiguousarray(b1[:, None]),
        "W2T": np.ascontiguousarray(W2.T).astype(mm_c_np),                  # [128,128]
        "b2_col": np.ascontiguousarray(b2[:, None]),
        "W3T": np.ascontiguousarray(W3.T).astype(mm_c_np),                  # [128,256]
        "b3_col": np.ascontiguousarray(b3.reshape(2, 128, 1)),
        "iota": np.tile(np.arange(128, dtype=np.float32), (128, 1)),
    }
    return com


# ----------------------------------------------------------------------------
# kernel builder
# ----------------------------------------------------------------------------

def _build(tiles_pb, dtcfg):
    mm_a, h_dt, mm_c = DT_CFG[dtcfg]
    nt = sum(t for pair in tiles_pb for t in pair)
    Relu = mybir.ActivationFunctionType.Relu
    Sig = mybir.ActivationFunctionType.Sigmoid
    Tanh = mybir.ActivationFunctionType.Tanh

    nc = bacc.Bacc("TRN2", target_bir_lowering=False, debug=False)

    featT_d = nc.dram_tensor("featT", [D, NPAD], mm_a, kind="ExternalInput")
    W_rT_d = nc.dram_tensor("W_rT", [NUM_REL, D, D], mm_a, kind="ExternalInput")
    b_r_col_d = nc.dram_tensor("b_r_col", [NUM_REL, D, 1], f32, kind="ExternalInput")
    b_r_row_d = nc.dram_tensor("b_r_row", [NUM_REL, 1, D], mm_a, kind="ExternalInput")
    W_ihT_d = nc.dram_tensor("W_ihT", [D, 768], mm_c, kind="ExternalInput")
    b_g_row_d = nc.dram_tensor("b_g_row", [1, 768], mm_c, kind="ExternalInput")
    W1T_d = nc.dram_tensor("W1T", [D, 128], mm_c, kind="ExternalInput")
    b1_col_d = nc.dram_tensor("b1_col", [128, 1], f32, kind="ExternalInput")
    W2T_d = nc.dram_tensor("W2T", [128, 128], mm_c, kind="ExternalInput")
    b2_col_d = nc.dram_tensor("b2_col", [128, 1], f32, kind="ExternalInput")
    W3T_d = nc.dram_tensor("W3T", [128, D_OUT], mm_c, kind="ExternalInput")
    b3_col_d = nc.dram_tensor("b3_col", [2, 128, 1], f32, kind="ExternalInput")
    iota_d = nc.dram_tensor("iota", [128, 128], f32, kind="ExternalInput")
    idxs_d = nc.dram_tensor("idxs16", [128, nt * 8], i16, kind="ExternalInput")
    dloc_d = nc.dram_tensor("dloc", [128, nt], f32, kind="ExternalInput")

    outT_d = nc.dram_tensor("outT", [D_OUT, NPC], f32, kind="ExternalOutput")

    H_d = [nc.dram_tensor(f"Htab{g}", [NUM_REL * NH, D], h_dt) for g in range(2)]

    with tile.TileContext(nc) as tc:
        with (
            tc.tile_pool(name="const", bufs=1) as cp,
            tc.tile_pool(name="work", bufs=3) as wp,
        ):
            # ---- load constants to SBUF ----
            W_rT_sb = {}
            for r in range(NUM_REL):
                for h in range(2):
                    t = cp.tile([128, D], mm_a, tag=f"wrt{r}{h}")
                    nc.sync.dma_start(t[:], W_rT_d[r, h * 128:(h + 1) * 128, :])
                    W_rT_sb[r, h] = t
            b_r_col_sb = {}
            for r in range(NUM_REL):
                for h in range(2):
                    t = cp.tile([128, 1], f32, tag=f"brc{r}{h}")
                    nc.sync.dma_start(t[:], b_r_col_d[r, h * 128:(h + 1) * 128, :])
                    b_r_col_sb[r, h] = t
            b_r_row_sb = {}
            for r in range(NUM_REL):
                t = cp.tile([1, D], mm_a, tag=f"brr{r}")
                nc.sync.dma_start(t[:], b_r_row_d[r, :, :])
                b_r_row_sb[r] = t
            W_ihT_sb = {}
            for h in range(2):
                t = cp.tile([128, 768], mm_c, tag=f"wih{h}")
                nc.sync.dma_start(t[:], W_ihT_d[h * 128:(h + 1) * 128, :])
                W_ihT_sb[h] = t
            b_g_row_sb = cp.tile([1, 768], mm_c, tag="bg")
            nc.sync.dma_start(b_g_row_sb[:], b_g_row_d[:, :])
            W1T_sb = {}
            for h in range(2):
                t = cp.tile([128, 128], mm_c, tag=f"w1t{h}")
                nc.sync.dma_start(t[:], W1T_d[h * 128:(h + 1) * 128, :])
                W1T_sb[h] = t
            b1_col_sb = cp.tile([128, 1], f32, tag="b1")
            nc.sync.dma_start(b1_col_sb[:], b1_col_d[:, :])
            W2T_sb = cp.tile([128, 128], mm_c, tag="w2t")
            nc.sync.dma_start(W2T_sb[:], W2T_d[:, :])
            b2_col_sb = cp.tile([128, 1], f32, tag="b2")
            nc.sync.dma_start(b2_col_sb[:], b2_col_d[:, :])
            W3T_sb = cp.tile([128, D_OUT], mm_c, tag="w3t")
            nc.sync.dma_start(W3T_sb[:], W3T_d[:, :])
            b3_col_sb = {}
            for h in range(2):
                t = cp.tile([128, 1], f32, tag=f"b3{h}")
                nc.sync.dma_start(t[:], b3_col_d[h, :, :])
                b3_col_sb[h] = t
            iota_sb = cp.tile([128, 128], f32, tag="iota")
            nc.sync.dma_start(iota_sb[:], iota_d[:, :])
            idxs_sb = cp.tile([128, nt * 8], i16, tag="idxs")
            nc.sync.dma_start(idxs_sb[:], idxs_d[:, :])
            dloc_sb = cp.tile([128, nt], f32, tag="dloc")
            nc.sync.dma_start(dloc_sb[:], dloc_d[:, :])
            ident = cp.tile([128, 128], f32, tag="ident")
            make_identity(nc, ident[:])
            ones_a = cp.tile([1, 128], mm_a, tag="onesa")
            nc.gpsimd.memset(ones_a[:], 1.0)
            ones_c = cp.tile([1, 128], mm_c, tag="onesc")
            nc.gpsimd.memset(ones_c[:], 1.0)

            # ---- phase A: build H tables (lo half then hi half) ----
            h_writes = [[], []]
            with tc.tile_pool(name="psA", bufs=1, space="PSUM") as psA:
                for g in range(2):
                    for r in range(NUM_REL):
                        for ntl in range(NT_H):
                            c0 = g * NH + ntl * 512
                            ft = {}
                            for h in range(2):
                                t = wp.tile([128, 512], mm_a, tag=f"ft{h}")
                                nc.sync.dma_start(
                                    t[:], featT_d[h * 128:(h + 1) * 128, c0:c0 + 512])
                                ft[h] = t
                            z1s = {}
                            for do_h in range(2):
                                z1p = psA.tile([128, 512], f32, tag=f"z1_{do_h}",
                                               space="PSUM", bufs=2)
                                for di_h in range(2):
                                    nc.tensor.matmul(
                                        z1p[:],
                                        lhsT=W_rT_sb[r, di_h][:, do_h * 128:(do_h + 1) * 128],
                                        rhs=ft[di_h][:],
                                        start=(di_h == 0), stop=(di_h == 1))
                                z = wp.tile([128, 512], mm_a, tag=f"z1s{do_h}")
                                nc.scalar.activation(z[:], z1p[:], Relu,
                                                     bias=b_r_col_sb[r, do_h][:],
                                                     scale=1.0)
                                z1s[do_h] = z
                            hs = wp.tile([128, 4, D], h_dt, tag="hs", bufs=2)
                            for c4 in range(4):
                                hp = psA.tile([128, D], f32, tag="hp",
                                              space="PSUM", bufs=2)
                                sl = slice(c4 * 128, (c4 + 1) * 128)
                                nc.tensor.matmul(hp[:], lhsT=z1s[0][:, sl],
                                                 rhs=W_rT_sb[r, 0][:],
                                                 start=True, stop=False)
                                nc.tensor.matmul(hp[:], lhsT=z1s[1][:, sl],
                                                 rhs=W_rT_sb[r, 1][:],
                                                 start=False, stop=False)
                                nc.tensor.matmul(hp[:], lhsT=ones_a[:],
                                                 rhs=b_r_row_sb[r][:],
                                                 start=False, stop=True)
                                nc.scalar.activation(hs[:, c4, :], hp[:], Relu,
                                                     bias=0.0, scale=1.0)
                            row0 = r * NH + ntl * 512
                            w = nc.sync.dma_start(
                                H_d[g][row0:row0 + 512, :].rearrange(
                                    "(c p) d -> p c d", p=128),
                                hs[:])
                            h_writes[g].append(w.ins)

            # ---- phase B + C per dst block ----
            grp_ofs = {}
            acc = 0
            for b in range(NBLK):
                for g in range(2):
                    grp_ofs[b, g] = acc
                    acc += tiles_pb[b][g]
            with tc.tile_pool(name="psB", bufs=1, space="PSUM") as psB:
                for b in range(NBLK):
                    aggp = psB.tile([128, D], f32, tag="agg", space="PSUM", bufs=2)
                    total_tiles = tiles_pb[b][0] + tiles_pb[b][1]
                    done = 0
                    for g in range(2):
                        tb = tiles_pb[b][g]
                        if tb == 0:
                            continue
                        ofs = grp_ofs[b, g]
                        msup = wp.tile([128, tb, D], h_dt, tag=f"msup{g}", bufs=2)
                        g_inst = nc.gpsimd.dma_gather(
                            out_ap=msup[:], in_ap=H_d[g][:],
                            idxs_ap=idxs_sb[:, ofs * 8:(ofs + tb) * 8],
                            num_idxs=tb * 128, num_idxs_reg=tb * 128,
                            elem_size=D, single_packet=False)
                        # dma_gather's DRAM read is not tracked by tile deps;
                        # order it after this half's H writes.
                        for w in h_writes[g]:
                            add_dep_helper(g_inst.ins, w,
                                           reason="gather waits on Htab writes")
                        selsup = wp.tile([128, tb, 128], h_dt, tag=f"selsup{g}",
                                         bufs=2)
                        for t in range(tb):
                            nc.vector.tensor_tensor(
                                out=selsup[:, t, :],
                                in0=dloc_sb[:, ofs + t:ofs + t + 1].to_broadcast(
                                    [128, 128]),
                                in1=iota_sb[:],
                                op=mybir.AluOpType.is_equal)
                        for t in range(tb):
                            nc.tensor.matmul(aggp[:], lhsT=selsup[:, t, :],
                                             rhs=msup[:, t, :],
                                             start=(done == 0),
                                             stop=(done == total_tiles - 1))
                            done += 1

                    # --- phase C ---
                    nn = min(128, NPC - b * 128)
                    agg_sb = wp.tile([128, D], f32, tag="aggsb")
                    nc.vector.tensor_copy(agg_sb[:], aggp[:])
                    aggT = {}
                    for h in range(2):
                        tp = psB.tile([128, 128], f32, tag="tp", space="PSUM",
                                      bufs=2)
                        nc.tensor.transpose(tp[:], agg_sb[:, h * 128:(h + 1) * 128],
                                            ident[:])
                        at = wp.tile([128, 128], mm_c, tag=f"aggT{h}")
                        nc.vector.tensor_copy(at[:], tp[:])
                        aggT[h] = at
                    # gates: bank0 = i (256 cols), bank1 = g|o (512 cols)
                    g0 = psB.tile([128, 256], f32, tag="g0", space="PSUM")
                    g1 = psB.tile([128, 512], f32, tag="g1", space="PSUM")
                    for h in range(2):
                        nc.tensor.matmul(g0[:], lhsT=aggT[h][:],
                                         rhs=W_ihT_sb[h][:, 0:256],
                                         start=(h == 0), stop=False)
                    nc.tensor.matmul(g0[:], lhsT=ones_c[:], rhs=b_g_row_sb[:, 0:256],
                                     start=False, stop=True)
                    for h in range(2):
                        nc.tensor.matmul(g1[:], lhsT=aggT[h][:],
                                         rhs=W_ihT_sb[h][:, 256:768],
                                         start=(h == 0), stop=False)
                    nc.tensor.matmul(g1[:], lhsT=ones_c[:], rhs=b_g_row_sb[:, 256:768],
                                     start=False, stop=True)
                    si = wp.tile([128, 256], f32, tag="si")
                    nc.scalar.activation(si[:], g0[:], Sig, bias=0.0, scale=1.0)
                    tg = wp.tile([128, 256], f32, tag="tg")
                    nc.scalar.activation(tg[:], g1[:, 0:256], Tanh, bias=0.0, scale=1.0)
                    so = wp.tile([128, 256], f32, tag="so")
                    nc.scalar.activation(so[:], g1[:, 256:512], Sig, bias=0.0, scale=1.0)
                    cc = wp.tile([128, 256], f32, tag="cc")
                    nc.vector.tensor_mul(cc[:], si[:], tg[:])
                    tcc = wp.tile([128, 256], f32, tag="tcc")
                    nc.scalar.activation(tcc[:], cc[:], Tanh, bias=0.0, scale=1.0)
                    hn = wp.tile([128, 256], f32, tag="hn")
                    nc.vector.tensor_mul(hn[:], so[:], tcc[:])
                    hnT = {}
                    for h in range(2):
                        tp = psB.tile([128, 128], f32, tag="tp", space="PSUM",
                                      bufs=2)
                        nc.tensor.transpose(tp[:], hn[:, h * 128:(h + 1) * 128],
                                            ident[:])
                        ht = wp.tile([128, 128], mm_c, tag=f"hnT{h}")
                        nc.vector.tensor_copy(ht[:], tp[:])
                        hnT[h] = ht
                    # MLP (transposed activation layout: [feature, node])
                    x12 = psB.tile([128, 256], f32, tag="x12", space="PSUM")
                    for h in range(2):
                        nc.tensor.matmul(x12[:, 0:128], lhsT=W1T_sb[h][:],
                                         rhs=hnT[h][:],
                                         start=(h == 0), stop=(h == 1))
                    x1s = wp.tile([128, 128], mm_c, tag="x1s")
                    nc.scalar.activation(x1s[:], x12[:, 0:128], Relu,
                                         bias=b1_col_sb[:], scale=1.0)
                    nc.tensor.matmul(x12[:, 128:256], lhsT=W2T_sb[:], rhs=x1s[:],
                                     start=True, stop=True)
                    x2s = wp.tile([128, 128], mm_c, tag="x2s")
                    nc.scalar.activation(x2s[:], x12[:, 128:256], Relu,
                                         bias=b2_col_sb[:], scale=1.0)
                    outp = psB.tile([128, 256], f32, tag="outp", space="PSUM")
                    for oh in range(2):
                        nc.tensor.matmul(outp[:, oh * 128:(oh + 1) * 128],
                                         lhsT=W3T_sb[:, oh * 128:(oh + 1) * 128],
                                         rhs=x2s[:], start=True, stop=True)
                    for oh in range(2):
                        osb = wp.tile([128, 128], f32, tag=f"osb{oh}")
                        nc.vector.tensor_scalar_add(
                            osb[:], outp[:, oh * 128:(oh + 1) * 128],
                            b3_col_sb[oh][:])
                        nc.sync.dma_start(
                            outT_d[oh * 128:(oh + 1) * 128, b * 128:b * 128 + nn],
                            osb[:, 0:nn])

    nc.compile()
    return nc


_CACHE = {}


def _get_nc(tiles_key, dtcfg):
    key = (tiles_key, dtcfg)
    if key not in _CACHE:
        _CACHE[key] = _build([list(p) for p in tiles_key], dtcfg)
    return _CACHE[key]


# ----------------------------------------------------------------------------
# public entry
# ----------------------------------------------------------------------------

def kernel(dtcfg="bf16", **inputs) -> np.ndarray:
    src = np.asarray(inputs["src"], dtype=np.int32)
    dst = np.asarray(inputs["dst"], dtype=np.int32)
    rel = np.asarray(inputs["rel"], dtype=np.int32)

    mm_a, h_dt, mm_c = DT_CFG[dtcfg]
    com = _prep_weights(inputs, _np_dt[mm_a], _np_dt[mm_c])
    tiles_pb, idxs16_all, dloc_all = _prep_edges(src, dst, rel)

    nc = _get_nc(tuple(tuple(p) for p in tiles_pb), dtcfg)

    in_maps = []
    for c in range(NCORES):
        m = dict(com)
        m["idxs16"] = np.ascontiguousarray(idxs16_all[c])
        m["dloc"] = np.ascontiguousarray(dloc_all[c])
        in_maps.append(m)

    res = bass_utils.run_bass_kernel_spmd(nc, in_maps, core_ids=list(range(NCORES)))

    out = np.empty((N_NODES, D_OUT), dtype=np.float32)
    for c in range(NCORES):
        out[c * NPC:(c + 1) * NPC, :] = res.results[c]["outT"].T
    return out


# revision 20
# speedup vs baseline: 1.3931x; 1.0209x over previous
"""Trainium2 Bass kernel for a relational GCN layer (message passing + LSTM + MLP).

Math (reference):
  S = feat[src]                               # [E, D] gather
  msgs[e] = edge_nn(S[e], W_rel[rel[e]])      # tied 2-layer relu MLP per relation
  agg = segment_sum(msgs, dst, N)             # [N, D]
  hn = LSTM-step(agg) from zero state         # [N, D]
  out = MLP(hn)                               # [N, D_OUT]

Key algebraic optimization: msgs[e] depends only on (rel[e], src[e]) of which
there are NUM_REL*N = 20k combos << E = 320k.  So precompute the transformed
node table H[r, s] = edge_nn(feat[s], W_rel[r]) once (dense GEMMs), then the
whole edge phase collapses to row-gather + segment-sum.

Distribution: edges are sharded by DESTINATION node range across the 8 cores
(core c owns dst in [1250c, 1250(c+1))), so every core computes complete
aggregates for its own 1250 nodes and no cross-core communication is needed
(on-chip collectives run at ~60 GB/s — slower than recomputing H per core).

The SWDGE gather descriptor generation (~8 ns/row, serialized on the GPSIMD
engine) is the hard floor of the edge phase, so the kernel is organised to
hide everything else behind it:
  phase A: H table built in 4 src-quarters written to separate HBM tensors;
           the gather chain for quarter q starts as soon as quarter q's H is
           written, overlapping the remaining PE work.
  phase B: gathers run quarter-major: for each q, one batched dma_gather per
           dst block pulls that block's quarter-q messages; a one-hot
           selector (DVE is_equal vs an iota row) feeds segment-sum matmuls
           into a rotating PSUM bank, which is then added into a per-block
           SBUF accumulator (only 1-2 PSUM banks needed).
  phase C: after the last quarter, per block: LSTM + MLP (activations kept
           transposed so weights stay stationary and biases land on
           partitions), output written transposed; host reassembles.
"""

import math
import numpy as np
import ml_dtypes

import concourse.bacc as bacc
import concourse.bass as bass
import concourse.mybir as mybir
import concourse.tile as tile
from concourse import bass_utils
from concourse.masks import make_identity
from concourse.tile import add_dep_helper

# ---- problem constants (hardcoded per spec) ----
N_NODES = 10000
N_EDGES = 320000
D = 256
D_OUT = 256
NUM_REL = 2
NCORES = 8
NPC = N_NODES // NCORES          # 1250 nodes per core
NBLK = math.ceil(NPC / 128)      # 10 dst blocks per core (last has 98 rows)
NPAD = 10240                     # node count padded to 20 tiles of 512
NQ = 4                           # src quarters
NHQ = NPAD // NQ                 # 2560 nodes per quarter
NT_Q = NHQ // 512                # 5 node tiles per quarter (per relation)

f32 = mybir.dt.float32
bf16 = mybir.dt.bfloat16
i16 = mybir.dt.int16

# dtype config: (phase-A matmul dt, H table dt, phase-C matmul dt)
DT_CFG = {
    "bf16": (bf16, bf16, bf16),
    "f32": (f32, f32, f32),
}
_np_dt = {bf16: ml_dtypes.bfloat16, f32: np.float32}

PAD_DLOC = 999.0  # sentinel: one-hot selector row all-zero -> edge ignored


# ----------------------------------------------------------------------------
# host-side preprocessing
# ----------------------------------------------------------------------------

def _prep_edges(src, dst, rel):
    """Shard edges by dst range; group per (dst-block, src-quarter); build the
    dma_gather index array and in-block dst-offset array with a static tiling
    common to all cores.

    Edge i of a group lands at msup[i % 128, i // 128, :] (dma_gather layout),
    so its gather index goes to idxs16[i % 16 (+16g), col*8 + i // 16] and its
    dst offset to dloc[i % 128, col + i // 128].
    """
    q = src // NHQ                       # 0..3
    core = dst // NPC
    blk = (dst % NPC) // 128
    group_key = (core * NBLK + blk) * NQ + q
    order = np.argsort(group_key, kind="stable")
    dst_s = dst[order]
    q_s = q[order]
    row_s = (rel[order].astype(np.int64) * NHQ + (src[order] % NHQ)).astype(np.int16)

    core_s = dst_s // NPC
    blk_s = (dst_s % NPC) // 128
    dloc_s = ((dst_s % NPC) % 128).astype(np.float32)

    counts = np.zeros((NCORES, NBLK, NQ), dtype=np.int64)
    np.add.at(counts, (core_s, blk_s, q_s), 1)
    tiles_pb = [[int(math.ceil(counts[:, b, g].max() / 128)) for g in range(NQ)]
                for b in range(NBLK)]
    nt = sum(t for row in tiles_pb for t in row)
    grp_ofs = {}
    acc = 0
    for g in range(NQ):          # quarter-major column layout
        for b in range(NBLK):
            grp_ofs[b, g] = acc
            acc += tiles_pb[b][g]

    idxs16_all = np.zeros((NCORES, 16, nt * 8), dtype=np.int16)
    dloc_all = np.full((NCORES, 128, nt), PAD_DLOC, dtype=np.float32)

    flat_counts = counts.reshape(-1)
    flat_starts = np.concatenate([[0], np.cumsum(flat_counts)[:-1]])
    starts = flat_starts.reshape(NCORES, NBLK, NQ)

    for c in range(NCORES):
        for b in range(NBLK):
            for g in range(NQ):
                n = int(counts[c, b, g])
                if n == 0:
                    continue
                s = int(starts[c, b, g])
                k = np.arange(n)
                ofs = grp_ofs[b, g]
                idxs16_all[c, k % 16, ofs * 8 + k // 16] = row_s[s:s + n]
                dloc_all[c, k % 128, ofs + k // 128] = dloc_s[s:s + n]
    idxs16_all = np.tile(idxs16_all, (1, 8, 1))
    return tiles_pb, idxs16_all, dloc_all


def _prep_weights(inputs, mm_a_np, mm_c_np):
    feat = np.asarray(inputs["feat"], dtype=np.float32)
    W_rel = np.asarray(inputs["W_rel"], dtype=np.float32)
    b_rel = np.asarray(inputs["b_rel"], dtype=np.float32)
    W_ih = np.asarray(inputs["W_ih"], dtype=np.float32)
    b_ih = np.asarray(inputs["b_ih"], dtype=np.float32)
    b_hh = np.asarray(inputs["b_hh"], dtype=np.float32)
    W1 = np.asarray(inputs["W1"], dtype=np.float32)
    b1 = np.asarray(inputs["b1"], dtype=np.float32)
    W2 = np.asarray(inputs["W2"], dtype=np.float32)
    b2 = np.asarray(inputs["b2"], dtype=np.float32)
    W3 = np.asarray(inputs["W3"], dtype=np.float32)
    b3 = np.asarray(inputs["b3"], dtype=np.float32)

    featT = np.zeros((D, NPAD), dtype=np.float32)
    featT[:, :N_NODES] = feat.T
    keep = np.r_[0:256, 512:1024]  # i, g, o gate columns (f unused: c0 = 0)
    com = {
        "featT": featT.astype(mm_a_np),
        "W_rT": np.ascontiguousarray(np.transpose(W_rel, (0, 2, 1))).astype(mm_a_np),
        "b_r_col": np.ascontiguousarray(b_rel[:, :, None]),                 # f32
        "b_r_rep": np.ascontiguousarray(
            np.broadcast_to(b_rel[:, None, :], (NUM_REL, 128, D))).copy(),  # f32
        "W_ihT": np.ascontiguousarray(W_ih.T[:, keep]).astype(mm_c_np),     # [256,768]
        "b_g_row": np.ascontiguousarray((b_ih + b_hh)[keep][None, :]).astype(mm_c_np),
        "W1T": np.ascontiguousarray(W1.T).astype(mm_c_np),                  # [256,128]
        "b1_col": np.ascontiguousarray(b1[:, None]),
        "W2T": np.ascontiguousarray(W2.T).astype(mm_c_np),                  # [128,128]
        "b2_col": np.ascontiguousarray(b2[:, None]),
        "W3T": np.ascontiguousarray(W3.T).astype(mm_c_np),                  # [128,256]
        "b3_col": np.ascontiguousarray(b3.reshape(2, 128, 1)),
        "iota": np.tile(np.arange(128, dtype=np.float32), (128, 1)),
    }
    return com


# ----------------------------------------------------------------------------
# kernel builder
# ----------------------------------------------------------------------------

def _build(tiles_pb, dtcfg):
    mm_a, h_dt, mm_c = DT_CFG[dtcfg]
    nt = sum(t for row in tiles_pb for t in row)
    Relu = mybir.ActivationFunctionType.Relu
    Sig = mybir.ActivationFunctionType.Sigmoid
    Tanh = mybir.ActivationFunctionType.Tanh

    nc = bacc.Bacc("TRN2", target_bir_lowering=False, debug=False)

    featT_d = nc.dram_tensor("featT", [D, NPAD], mm_a, kind="ExternalInput")
    W_rT_d = nc.dram_tensor("W_rT", [NUM_REL, D, D], mm_a, kind="ExternalInput")
    b_r_col_d = nc.dram_tensor("b_r_col", [NUM_REL, D, 1], f32, kind="ExternalInput")
    b_r_rep_d = nc.dram_tensor("b_r_rep", [NUM_REL, 128, D], f32, kind="ExternalInput")
    W_ihT_d = nc.dram_tensor("W_ihT", [D, 768], mm_c, kind="ExternalInput")
    b_g_row_d = nc.dram_tensor("b_g_row", [1, 768], mm_c, kind="ExternalInput")
    W1T_d = nc.dram_tensor("W1T", [D, 128], mm_c, kind="ExternalInput")
    b1_col_d = nc.dram_tensor("b1_col", [128, 1], f32, kind="ExternalInput")
    W2T_d = nc.dram_tensor("W2T", [128, 128], mm_c, kind="ExternalInput")
    b2_col_d = nc.dram_tensor("b2_col", [128, 1], f32, kind="ExternalInput")
    W3T_d = nc.dram_tensor("W3T", [128, D_OUT], mm_c, kind="ExternalInput")
    b3_col_d = nc.dram_tensor("b3_col", [2, 128, 1], f32, kind="ExternalInput")
    iota_d = nc.dram_tensor("iota", [128, 128], f32, kind="ExternalInput")
    idxs_d = nc.dram_tensor("idxs16", [128, nt * 8], i16, kind="ExternalInput")
    dloc_d = nc.dram_tensor("dloc", [128, nt], f32, kind="ExternalInput")

    outT_d = nc.dram_tensor("outT", [D_OUT, NPC], f32, kind="ExternalOutput")

    H_d = [nc.dram_tensor(f"Htab{g}", [NUM_REL * NHQ, D], h_dt) for g in range(NQ)]

    with tile.TileContext(nc) as tc:
        with (
            tc.tile_pool(name="const", bufs=1) as cp,
            tc.tile_pool(name="work", bufs=3) as wp,
            tc.tile_pool(name="aggpool", bufs=1) as ap_pool,
            tc.tile_pool(name="psum", bufs=1, space="PSUM") as ps,
        ):
            # ---- load constants to SBUF ----
            W_rT_sb = {}
            for r in range(NUM_REL):
                for h in range(2):
                    t = cp.tile([128, D], mm_a, tag=f"wrt{r}{h}")
                    nc.sync.dma_start(t[:], W_rT_d[r, h * 128:(h + 1) * 128, :])
                    W_rT_sb[r, h] = t
            b_r_col_sb = {}
            for r in range(NUM_REL):
                for h in range(2):
                    t = cp.tile([128, 1], f32, tag=f"brc{r}{h}")
                    nc.sync.dma_start(t[:], b_r_col_d[r, h * 128:(h + 1) * 128, :])
                    b_r_col_sb[r, h] = t
            b_r_rep_sb = {}
            for r in range(NUM_REL):
                t = cp.tile([128, D], f32, tag=f"brr{r}")
                nc.sync.dma_start(t[:], b_r_rep_d[r, :, :])
                b_r_rep_sb[r] = t
            W_ihT_sb = {}
            for h in range(2):
                t = cp.tile([128, 768], mm_c, tag=f"wih{h}")
                nc.sync.dma_start(t[:], W_ihT_d[h * 128:(h + 1) * 128, :])
                W_ihT_sb[h] = t
            b_g_row_sb = cp.tile([1, 768], mm_c, tag="bg")
            nc.sync.dma_start(b_g_row_sb[:], b_g_row_d[:, :])
            W1T_sb = {}
            for h in range(2):
                t = cp.tile([128, 128], mm_c, tag=f"w1t{h}")
                nc.sync.dma_start(t[:], W1T_d[h * 128:(h + 1) * 128, :])
                W1T_sb[h] = t
            b1_col_sb = cp.tile([128, 1], f32, tag="b1")
            nc.sync.dma_start(b1_col_sb[:], b1_col_d[:, :])
            W2T_sb = cp.tile([128, 128], mm_c, tag="w2t")
            nc.sync.dma_start(W2T_sb[:], W2T_d[:, :])
            b2_col_sb = cp.tile([128, 1], f32, tag="b2")
            nc.sync.dma_start(b2_col_sb[:], b2_col_d[:, :])
            W3T_sb = cp.tile([128, D_OUT], mm_c, tag="w3t")
            nc.sync.dma_start(W3T_sb[:], W3T_d[:, :])
            b3_col_sb = {}
            for h in range(2):
                t = cp.tile([128, 1], f32, tag=f"b3{h}")
                nc.sync.dma_start(t[:], b3_col_d[h, :, :])
                b3_col_sb[h] = t
            iota_sb = cp.tile([128, 128], f32, tag="iota")
            nc.sync.dma_start(iota_sb[:], iota_d[:, :])
            idxs_sb = cp.tile([128, nt * 8], i16, tag="idxs")
            nc.sync.dma_start(idxs_sb[:], idxs_d[:, :])
            dloc_sb = cp.tile([128, nt], f32, tag="dloc")
            nc.sync.dma_start(dloc_sb[:], dloc_d[:, :])
            ident = cp.tile([128, 128], f32, tag="ident")
            make_identity(nc, ident[:])
            ones_c = cp.tile([1, 128], mm_c, tag="onesc")
            nc.gpsimd.memset(ones_c[:], 1.0)

            # per-block SBUF aggregators
            agg_sb = {}
            for b in range(NBLK):
                agg_sb[b] = ap_pool.tile([128, D], f32, tag=f"agg{b}", name=f"agg{b}")

            grp_ofs = {}
            acc = 0
            for g in range(NQ):
                for b in range(NBLK):
                    grp_ofs[b, g] = acc
                    acc += tiles_pb[b][g]

            # ---- phase A (per quarter) interleaved with phase B gathers ----
            h_writes = [[] for _ in range(NQ)]
            for q in range(NQ):
                for r in range(NUM_REL):
                    for ntl in range(NT_Q):
                        c0 = q * NHQ + ntl * 512
                        ft = {}
                        for h in range(2):
                            t = wp.tile([128, 512], mm_a, tag=f"ft{h}")
                            nc.sync.dma_start(
                                t[:], featT_d[h * 128:(h + 1) * 128, c0:c0 + 512])
                            ft[h] = t
                        z1s = {}
                        for do_h in range(2):
                            z1p = ps.tile([128, 512], f32, tag=f"z1_{do_h}",
                                          space="PSUM", bufs=1)
                            for di_h in range(2):
                                nc.tensor.matmul(
                                    z1p[:],
                                    lhsT=W_rT_sb[r, di_h][:, do_h * 128:(do_h + 1) * 128],
                                    rhs=ft[di_h][:],
                                    start=(di_h == 0), stop=(di_h == 1))
                            z = wp.tile([128, 512], mm_a, tag=f"z1s{do_h}")
                            nc.scalar.activation(z[:], z1p[:], Relu,
                                                 bias=b_r_col_sb[r, do_h][:],
                                                 scale=1.0)
                            z1s[do_h] = z
                        hs = wp.tile([128, 4, D], h_dt, tag="hs", bufs=2)
                        for c4 in range(4):
                            hp = ps.tile([128, D], f32, tag="hp",
                                         space="PSUM", bufs=2)
                            sl = slice(c4 * 128, (c4 + 1) * 128)
                            nc.tensor.matmul(hp[:], lhsT=z1s[0][:, sl],
                                             rhs=W_rT_sb[r, 0][:],
                                             start=True, stop=False)
                            nc.tensor.matmul(hp[:], lhsT=z1s[1][:, sl],
                                             rhs=W_rT_sb[r, 1][:],
                                             start=False, stop=True)
                            # bias (free-dim) + relu: DVE add then ACT relu+cast
                            nc.vector.tensor_add(hp[:], hp[:], b_r_rep_sb[r][:])
                            nc.scalar.activation(hs[:, c4, :], hp[:], Relu,
                                                 bias=0.0, scale=1.0)
                        row0 = r * NHQ + ntl * 512
                        w = nc.sync.dma_start(
                            H_d[q][row0:row0 + 512, :].rearrange(
                                "(c p) d -> p c d", p=128),
                            hs[:])
                        h_writes[q].append(w.ins)

            # ---- phase B: quarter-major gathers + segment-sum ----
            for q in range(NQ):
                for b in range(NBLK):
                    tb = tiles_pb[b][q]
                    if tb == 0:
                        continue
                    ofs = grp_ofs[b, q]
                    msup = wp.tile([128, tb, D], h_dt, tag="msup", bufs=3)
                    g_inst = nc.gpsimd.dma_gather(
                        out_ap=msup[:], in_ap=H_d[q][:],
                        idxs_ap=idxs_sb[:, ofs * 8:(ofs + tb) * 8],
                        num_idxs=tb * 128, num_idxs_reg=tb * 128,
                        elem_size=D, single_packet=False)
                    # dma_gather's DRAM read is not tracked by tile deps;
                    # order it after this quarter's H writes.
                    for w in h_writes[q]:
                        add_dep_helper(g_inst.ins, w,
                                       reason="gather waits on Htab writes")
                    selsup = wp.tile([128, tb, 128], h_dt, tag="selsup", bufs=3)
                    for t in range(tb):
                        nc.vector.tensor_tensor(
                            out=selsup[:, t, :],
                            in0=dloc_sb[:, ofs + t:ofs + t + 1].to_broadcast(
                                [128, 128]),
                            in1=iota_sb[:],
                            op=mybir.AluOpType.is_equal)
                    segp = ps.tile([128, D], f32, tag="seg", space="PSUM", bufs=2)
                    for t in range(tb):
                        nc.tensor.matmul(segp[:], lhsT=selsup[:, t, :],
                                         rhs=msup[:, t, :],
                                         start=(t == 0), stop=(t == tb - 1))
                    if q == 0:
                        nc.vector.tensor_copy(agg_sb[b][:], segp[:])
                    else:
                        nc.vector.tensor_add(agg_sb[b][:], agg_sb[b][:], segp[:])

                    # ---- phase C: after the last quarter's contribution ----
                    if q != NQ - 1:
                        continue
                    nn = min(128, NPC - b * 128)
                    # cb1: [0:256]=i gates, [256:512]=g then o (sequential)
                    # cb2: [0:128]=transpose scratch, [128:256]=x1,
                    #      [256:384]=x2, [384:512]=out halves (sequential)
                    cb1 = ps.tile([128, 512], f32, tag="cb1", space="PSUM")
                    cb2 = ps.tile([128, 512], f32, tag="cb2", space="PSUM")
                    aggT = {}
                    for h in range(2):
                        nc.tensor.transpose(cb2[:, 0:128],
                                            agg_sb[b][:, h * 128:(h + 1) * 128],
                                            ident[:])
                        at = wp.tile([128, 128], mm_c, tag=f"aggT{h}")
                        nc.vector.tensor_copy(at[:], cb2[:, 0:128])
                        aggT[h] = at
                    # i gates
                    for h in range(2):
                        nc.tensor.matmul(cb1[:, 0:256], lhsT=aggT[h][:],
                                         rhs=W_ihT_sb[h][:, 0:256],
                                         start=(h == 0), stop=False)
                    nc.tensor.matmul(cb1[:, 0:256], lhsT=ones_c[:],
                                     rhs=b_g_row_sb[:, 0:256],
                                     start=False, stop=True)
                    si = wp.tile([128, 256], f32, tag="si")
                    nc.scalar.activation(si[:], cb1[:, 0:256], Sig, bias=0.0,
                                         scale=1.0)
                    # g gates
                    for h in range(2):
                        nc.tensor.matmul(cb1[:, 256:512], lhsT=aggT[h][:],
                                         rhs=W_ihT_sb[h][:, 256:512],
                                         start=(h == 0), stop=False)
                    nc.tensor.matmul(cb1[:, 256:512], lhsT=ones_c[:],
                                     rhs=b_g_row_sb[:, 256:512],
                                     start=False, stop=True)
                    tg = wp.tile([128, 256], f32, tag="tg")
                    nc.scalar.activation(tg[:], cb1[:, 256:512], Tanh, bias=0.0,
                                         scale=1.0)
                    # o gates (reuse the same psum region)
                    for h in range(2):
                        nc.tensor.matmul(cb1[:, 256:512], lhsT=aggT[h][:],
                                         rhs=W_ihT_sb[h][:, 512:768],
                                         start=(h == 0), stop=False)
                    nc.tensor.matmul(cb1[:, 256:512], lhsT=ones_c[:],
                                     rhs=b_g_row_sb[:, 512:768],
                                     start=False, stop=True)
                    so = wp.tile([128, 256], f32, tag="so")
                    nc.scalar.activation(so[:], cb1[:, 256:512], Sig, bias=0.0,
                                         scale=1.0)
                    cc = wp.tile([128, 256], f32, tag="cc")
                    nc.vector.tensor_mul(cc[:], si[:], tg[:])
                    tcc = wp.tile([128, 256], f32, tag="tcc")
                    nc.scalar.activation(tcc[:], cc[:], Tanh, bias=0.0, scale=1.0)
                    hn = wp.tile([128, 256], f32, tag="hn")
                    nc.vector.tensor_mul(hn[:], so[:], tcc[:])
                    hnT = {}
                    for h in range(2):
                        nc.tensor.transpose(cb2[:, 0:128],
                                            hn[:, h * 128:(h + 1) * 128],
                                            ident[:])
                        ht = wp.tile([128, 128], mm_c, tag=f"hnT{h}")
                        nc.vector.tensor_copy(ht[:], cb2[:, 0:128])
                        hnT[h] = ht
                    # MLP (transposed activation layout: [feature, node])
                    for h in range(2):
                        nc.tensor.matmul(cb2[:, 128:256], lhsT=W1T_sb[h][:],
                                         rhs=hnT[h][:],
                                         start=(h == 0), stop=(h == 1))
                    x1s = wp.tile([128, 128], mm_c, tag="x1s")
                    nc.scalar.activation(x1s[:], cb2[:, 128:256], Relu,
                                         bias=b1_col_sb[:], scale=1.0)
                    nc.tensor.matmul(cb2[:, 256:384], lhsT=W2T_sb[:], rhs=x1s[:],
                                     start=True, stop=True)
                    x2s = wp.tile([128, 128], mm_c, tag="x2s")
                    nc.scalar.activation(x2s[:], cb2[:, 256:384], Relu,
                                         bias=b2_col_sb[:], scale=1.0)
                    for oh in range(2):
                        nc.tensor.matmul(cb2[:, 384:512],
                                         lhsT=W3T_sb[:, oh * 128:(oh + 1) * 128],
                                         rhs=x2s[:], start=True, stop=True)
                        osb = wp.tile([128, 128], f32, tag=f"osb{oh}")
                        nc.vector.tensor_scalar_add(
                            osb[:], cb2[:, 384:512], b3_col_sb[oh][:])
                        nc.sync.dma_start(
                            outT_d[oh * 128:(oh + 1) * 128, b * 128:b * 128 + nn],
                            osb[:, 0:nn])

    nc.compile()
    return nc


_CACHE = {}


def _get_nc(tiles_key, dtcfg):
    key = (tiles_key, dtcfg)
    if key not in _CACHE:
        _CACHE[key] = _build([list(p) for p in tiles_key], dtcfg)
    return _CACHE[key]


# ----------------------------------------------------------------------------
# public entry
# ----------------------------------------------------------------------------

def kernel(dtcfg="bf16", **inputs) -> np.ndarray:
    src = np.asarray(inputs["src"], dtype=np.int32)
    dst = np.asarray(inputs["dst"], dtype=np.int32)
    rel = np.asarray(inputs["rel"], dtype=np.int32)

    mm_a, h_dt, mm_c = DT_CFG[dtcfg]
    com = _prep_weights(inputs, _np_dt[mm_a], _np_dt[mm_c])
    tiles_pb, idxs16_all, dloc_all = _prep_edges(src, dst, rel)

    nc = _get_nc(tuple(tuple(p) for p in tiles_pb), dtcfg)

    in_maps = []
    for c in range(NCORES):
        m = dict(com)
        m["idxs16"] = np.ascontiguousarray(idxs16_all[c])
        m["dloc"] = np.ascontiguousarray(dloc_all[c])
        in_maps.append(m)

    res = bass_utils.run_bass_kernel_spmd(nc, in_maps, core_ids=list(range(NCORES)))

    out = np.empty((N_NODES, D_OUT), dtype=np.float32)
    for c in range(NCORES):
        out[c * NPC:(c + 1) * NPC, :] = res.results[c]["outT"].T
    return out


# revision 22
# speedup vs baseline: 1.5707x; 1.1276x over previous
"""Trainium2 Bass kernel for a relational GCN layer (message passing + LSTM + MLP).

Math (reference):
  S = feat[src]                               # [E, D] gather
  msgs[e] = edge_nn(S[e], W_rel[rel[e]])      # tied 2-layer relu MLP per relation
  agg = segment_sum(msgs, dst, N)             # [N, D]
  hn = LSTM-step(agg) from zero state         # [N, D]
  out = MLP(hn)                               # [N, D_OUT]

Key algebraic optimization: msgs[e] depends only on (rel[e], src[e]) of which
there are NUM_REL*N = 20k combos << E = 320k.  So precompute the transformed
node table H[r, s] = edge_nn(feat[s], W_rel[r]) once (dense GEMMs), then the
whole edge phase collapses to row-gather + segment-sum.

Distribution: edges are sharded by DESTINATION node range across the 8 cores
(core c owns dst in [1250c, 1250(c+1))), so every core computes complete
aggregates for its own 1250 nodes and no cross-core communication is needed
(on-chip collectives run at ~60 GB/s — slower than recomputing H per core).

The SWDGE gather descriptor generation (~8 ns/row, serialized on the GPSIMD
engine) is the hard floor of the edge phase, so the kernel is organised to
hide everything else behind it:
  phase A: H table built in 4 src-quarters written to separate HBM tensors;
           the gather chain for quarter q starts as soon as quarter q's H is
           written, overlapping the remaining PE work.
  phase B: gathers run quarter-major: for each q, one batched dma_gather per
           dst block pulls that block's quarter-q messages; a one-hot
           selector (DVE is_equal vs an iota row) feeds segment-sum matmuls
           into a rotating PSUM bank, which is then added into a per-block
           SBUF accumulator (only 1-2 PSUM banks needed).
  phase C: after the last quarter, per block: LSTM + MLP (activations kept
           transposed so weights stay stationary and biases land on
           partitions), output written transposed; host reassembles.
"""

import math
import numpy as np
import ml_dtypes

import concourse.bacc as bacc
import concourse.bass as bass
import concourse.mybir as mybir
import concourse.tile as tile
from concourse import bass_utils
from concourse.masks import make_identity
from concourse.tile import add_dep_helper

# ---- problem constants (hardcoded per spec) ----
N_NODES = 10000
N_EDGES = 320000
D = 256
D_OUT = 256
NUM_REL = 2
NCORES = 8
NPC = N_NODES // NCORES          # 1250 nodes per core
NBLK = math.ceil(NPC / 128)      # 10 dst blocks per core (last has 98 rows)
NPAD = 10240                     # node count padded to 20 tiles of 512
# src buckets: small leading buckets so the gather chain starts early while
# the rest of the H table is still being computed
BUCKETS = [1024, 1024, 2048, 2048, 2048, 2048]
BUCKET_BASE = [0, 1024, 2048, 4096, 6144, 8192]
NQ = len(BUCKETS)

f32 = mybir.dt.float32
bf16 = mybir.dt.bfloat16
i16 = mybir.dt.int16

# dtype config: (phase-A matmul dt, H table dt, phase-C matmul dt)
DT_CFG = {
    "bf16": (bf16, bf16, bf16),
    "f32": (f32, f32, f32),
}
_np_dt = {bf16: ml_dtypes.bfloat16, f32: np.float32}

PAD_DLOC = 999.0  # sentinel: one-hot selector row all-zero -> edge ignored


# ----------------------------------------------------------------------------
# host-side preprocessing
# ----------------------------------------------------------------------------

def _prep_edges(src, dst, rel):
    """Shard edges by dst range; group per (dst-block, src-quarter); build the
    dma_gather index array and in-block dst-offset array with a static tiling
    common to all cores.

    Edge i of a group lands at msup[i % 128, i // 128, :] (dma_gather layout),
    so its gather index goes to idxs16[i % 16 (+16g), col*8 + i // 16] and its
    dst offset to dloc[i % 128, col + i // 128].
    """
    base = np.asarray(BUCKET_BASE, dtype=np.int64)
    sizes = np.asarray(BUCKETS, dtype=np.int64)
    q = np.searchsorted(base, src, side="right") - 1     # bucket id
    core = dst // NPC
    blk = (dst % NPC) // 128
    group_key = (core * NBLK + blk) * NQ + q
    order = np.argsort(group_key, kind="stable")
    dst_s = dst[order]
    q_s = q[order]
    row_s = (rel[order].astype(np.int64) * sizes[q[order]]
             + (src[order].astype(np.int64) - base[q[order]])).astype(np.int16)

    core_s = dst_s // NPC
    blk_s = (dst_s % NPC) // 128
    dloc_s = ((dst_s % NPC) % 128).astype(np.float32)

    counts = np.zeros((NCORES, NBLK, NQ), dtype=np.int64)
    np.add.at(counts, (core_s, blk_s, q_s), 1)
    tiles_pb = [[int(math.ceil(counts[:, b, g].max() / 128)) for g in range(NQ)]
                for b in range(NBLK)]
    nt = sum(t for row in tiles_pb for t in row)
    grp_ofs = {}
    acc = 0
    for g in range(NQ):          # quarter-major column layout
        for b in range(NBLK):
            grp_ofs[b, g] = acc
            acc += tiles_pb[b][g]

    idxs16_all = np.zeros((NCORES, 16, nt * 8), dtype=np.int16)
    dloc_all = np.full((NCORES, 128, nt), PAD_DLOC, dtype=np.float32)

    flat_counts = counts.reshape(-1)
    flat_starts = np.concatenate([[0], np.cumsum(flat_counts)[:-1]])
    starts = flat_starts.reshape(NCORES, NBLK, NQ)

    for c in range(NCORES):
        for b in range(NBLK):
            for g in range(NQ):
                n = int(counts[c, b, g])
                if n == 0:
                    continue
                s = int(starts[c, b, g])
                k = np.arange(n)
                ofs = grp_ofs[b, g]
                idxs16_all[c, k % 16, ofs * 8 + k // 16] = row_s[s:s + n]
                dloc_all[c, k % 128, ofs + k // 128] = dloc_s[s:s + n]
    idxs16_all = np.tile(idxs16_all, (1, 8, 1))
    return tiles_pb, idxs16_all, dloc_all


def _prep_weights(inputs, mm_a_np, mm_c_np):
    feat = np.asarray(inputs["feat"], dtype=np.float32)
    W_rel = np.asarray(inputs["W_rel"], dtype=np.float32)
    b_rel = np.asarray(inputs["b_rel"], dtype=np.float32)
    W_ih = np.asarray(inputs["W_ih"], dtype=np.float32)
    b_ih = np.asarray(inputs["b_ih"], dtype=np.float32)
    b_hh = np.asarray(inputs["b_hh"], dtype=np.float32)
    W1 = np.asarray(inputs["W1"], dtype=np.float32)
    b1 = np.asarray(inputs["b1"], dtype=np.float32)
    W2 = np.asarray(inputs["W2"], dtype=np.float32)
    b2 = np.asarray(inputs["b2"], dtype=np.float32)
    W3 = np.asarray(inputs["W3"], dtype=np.float32)
    b3 = np.asarray(inputs["b3"], dtype=np.float32)

    featT = np.zeros((D, NPAD), dtype=np.float32)
    featT[:, :N_NODES] = feat.T
    keep = np.r_[0:256, 512:1024]  # i, g, o gate columns (f unused: c0 = 0)
    com = {
        "featT": featT.astype(mm_a_np),
        "W_rT": np.ascontiguousarray(np.transpose(W_rel, (0, 2, 1))).astype(mm_a_np),
        "b_r_col": np.ascontiguousarray(b_rel[:, :, None]),                 # f32
        "b_r_rep": np.ascontiguousarray(
            np.broadcast_to(b_rel[:, None, :], (NUM_REL, 128, D))).copy(),  # f32
        "W_ihT": np.ascontiguousarray(W_ih.T[:, keep]).astype(mm_c_np),     # [256,768]
        "b_g_row": np.ascontiguousarray((b_ih + b_hh)[keep][None, :]).astype(mm_c_np),
        "W1T": np.ascontiguousarray(W1.T).astype(mm_c_np),                  # [256,128]
        "b1_col": np.ascontiguousarray(b1[:, None]),
        "W2T": np.ascontiguousarray(W2.T).astype(mm_c_np),                  # [128,128]
        "b2_col": np.ascontiguousarray(b2[:, None]),
        "W3T": np.ascontiguousarray(W3.T).astype(mm_c_np),                  # [128,256]
        "b3_col": np.ascontiguousarray(b3.reshape(2, 128, 1)),
        "iota": np.tile(np.arange(128, dtype=np.float32), (128, 1)),
    }
    return com


# ----------------------------------------------------------------------------
# kernel builder
# ----------------------------------------------------------------------------

def _build(tiles_pb, dtcfg):
    mm_a, h_dt, mm_c = DT_CFG[dtcfg]
    nt = sum(t for row in tiles_pb for t in row)
    Relu = mybir.ActivationFunctionType.Relu
    Sig = mybir.ActivationFunctionType.Sigmoid
    Tanh = mybir.ActivationFunctionType.Tanh

    nc = bacc.Bacc("TRN2", target_bir_lowering=False, debug=False)

    featT_d = nc.dram_tensor("featT", [D, NPAD], mm_a, kind="ExternalInput")
    W_rT_d = nc.dram_tensor("W_rT", [NUM_REL, D, D], mm_a, kind="ExternalInput")
    b_r_col_d = nc.dram_tensor("b_r_col", [NUM_REL, D, 1], f32, kind="ExternalInput")
    b_r_rep_d = nc.dram_tensor("b_r_rep", [NUM_REL, 128, D], f32, kind="ExternalInput")
    W_ihT_d = nc.dram_tensor("W_ihT", [D, 768], mm_c, kind="ExternalInput")
    b_g_row_d = nc.dram_tensor("b_g_row", [1, 768], mm_c, kind="ExternalInput")
    W1T_d = nc.dram_tensor("W1T", [D, 128], mm_c, kind="ExternalInput")
    b1_col_d = nc.dram_tensor("b1_col", [128, 1], f32, kind="ExternalInput")
    W2T_d = nc.dram_tensor("W2T", [128, 128], mm_c, kind="ExternalInput")
    b2_col_d = nc.dram_tensor("b2_col", [128, 1], f32, kind="ExternalInput")
    W3T_d = nc.dram_tensor("W3T", [128, D_OUT], mm_c, kind="ExternalInput")
    b3_col_d = nc.dram_tensor("b3_col", [2, 128, 1], f32, kind="ExternalInput")
    iota_d = nc.dram_tensor("iota", [128, 128], f32, kind="ExternalInput")
    idxs_d = nc.dram_tensor("idxs16", [128, nt * 8], i16, kind="ExternalInput")
    dloc_d = nc.dram_tensor("dloc", [128, nt], f32, kind="ExternalInput")

    outT_d = nc.dram_tensor("outT", [D_OUT, NPC], f32, kind="ExternalOutput")

    H_d = [nc.dram_tensor(f"Htab{g}", [NUM_REL * BUCKETS[g], D], h_dt)
           for g in range(NQ)]

    with tile.TileContext(nc) as tc:
        with (
            tc.tile_pool(name="const", bufs=1) as cp,
            tc.tile_pool(name="work", bufs=3) as wp,
            tc.tile_pool(name="aggpool", bufs=1) as ap_pool,
            tc.tile_pool(name="psA", bufs=1, space="PSUM") as psA,
            tc.tile_pool(name="psBC", bufs=1, space="PSUM") as psBC,
        ):
            # ---- load constants to SBUF ----
            W_rT_sb = {}
            for r in range(NUM_REL):
                for h in range(2):
                    t = cp.tile([128, D], mm_a, tag=f"wrt{r}{h}")
                    nc.sync.dma_start(t[:], W_rT_d[r, h * 128:(h + 1) * 128, :])
                    W_rT_sb[r, h] = t
            b_r_col_sb = {}
            for r in range(NUM_REL):
                for h in range(2):
                    t = cp.tile([128, 1], f32, tag=f"brc{r}{h}")
                    nc.sync.dma_start(t[:], b_r_col_d[r, h * 128:(h + 1) * 128, :])
                    b_r_col_sb[r, h] = t
            b_r_rep_sb = {}
            for r in range(NUM_REL):
                t = cp.tile([128, D], f32, tag=f"brr{r}")
                nc.sync.dma_start(t[:], b_r_rep_d[r, :, :])
                b_r_rep_sb[r] = t
            W_ihT_sb = {}
            for h in range(2):
                t = cp.tile([128, 768], mm_c, tag=f"wih{h}")
                nc.sync.dma_start(t[:], W_ihT_d[h * 128:(h + 1) * 128, :])
                W_ihT_sb[h] = t
            b_g_row_sb = cp.tile([1, 768], mm_c, tag="bg")
            nc.sync.dma_start(b_g_row_sb[:], b_g_row_d[:, :])
            W1T_sb = {}
            for h in range(2):
                t = cp.tile([128, 128], mm_c, tag=f"w1t{h}")
                nc.sync.dma_start(t[:], W1T_d[h * 128:(h + 1) * 128, :])
                W1T_sb[h] = t
            b1_col_sb = cp.tile([128, 1], f32, tag="b1")
            nc.sync.dma_start(b1_col_sb[:], b1_col_d[:, :])
            W2T_sb = cp.tile([128, 128], mm_c, tag="w2t")
            nc.sync.dma_start(W2T_sb[:], W2T_d[:, :])
            b2_col_sb = cp.tile([128, 1], f32, tag="b2")
            nc.sync.dma_start(b2_col_sb[:], b2_col_d[:, :])
            W3T_sb = cp.tile([128, D_OUT], mm_c, tag="w3t")
            nc.sync.dma_start(W3T_sb[:], W3T_d[:, :])
            b3_col_sb = {}
            for h in range(2):
                t = cp.tile([128, 1], f32, tag=f"b3{h}")
                nc.sync.dma_start(t[:], b3_col_d[h, :, :])
                b3_col_sb[h] = t
            iota_sb = cp.tile([128, 128], f32, tag="iota")
            nc.sync.dma_start(iota_sb[:], iota_d[:, :])
            idxs_sb = cp.tile([128, nt * 8], i16, tag="idxs")
            nc.sync.dma_start(idxs_sb[:], idxs_d[:, :])
            dloc_sb = cp.tile([128, nt], f32, tag="dloc")
            nc.sync.dma_start(dloc_sb[:], dloc_d[:, :])
            ident = cp.tile([128, 128], f32, tag="ident")
            make_identity(nc, ident[:])
            ones_c = cp.tile([1, 128], mm_c, tag="onesc")
            nc.gpsimd.memset(ones_c[:], 1.0)

            # per-block SBUF aggregators
            agg_sb = {}
            for b in range(NBLK):
                agg_sb[b] = ap_pool.tile([128, D], f32, tag=f"agg{b}", name=f"agg{b}")

            grp_ofs = {}
            acc = 0
            for g in range(NQ):
                for b in range(NBLK):
                    grp_ofs[b, g] = acc
                    acc += tiles_pb[b][g]

            # ---- phase A (per quarter) interleaved with phase B gathers ----
            h_writes = [[] for _ in range(NQ)]
            for q in range(NQ):
                for r in range(NUM_REL):
                    for ntl in range(BUCKETS[q] // 512):
                        c0 = BUCKET_BASE[q] + ntl * 512
                        ft = {}
                        for h in range(2):
                            t = wp.tile([128, 512], mm_a, tag=f"ft{h}")
                            nc.sync.dma_start(
                                t[:], featT_d[h * 128:(h + 1) * 128, c0:c0 + 512])
                            ft[h] = t
                        z1s = {}
                        for do_h in range(2):
                            z1p = psA.tile([128, 512], f32, tag="z1",
                                          space="PSUM", bufs=1)
                            for di_h in range(2):
                                nc.tensor.matmul(
                                    z1p[:],
                                    lhsT=W_rT_sb[r, di_h][:, do_h * 128:(do_h + 1) * 128],
                                    rhs=ft[di_h][:],
                                    start=(di_h == 0), stop=(di_h == 1))
                            z = wp.tile([128, 512], mm_a, tag=f"z1s{do_h}")
                            nc.scalar.activation(z[:], z1p[:], Relu,
                                                 bias=b_r_col_sb[r, do_h][:],
                                                 scale=1.0)
                            z1s[do_h] = z
                        hs = wp.tile([128, 4, D], h_dt, tag="hs", bufs=2)
                        for c4 in range(4):
                            hp = psA.tile([128, D], f32, tag="hp",
                                         space="PSUM", bufs=2)
                            sl = slice(c4 * 128, (c4 + 1) * 128)
                            nc.tensor.matmul(hp[:], lhsT=z1s[0][:, sl],
                                             rhs=W_rT_sb[r, 0][:],
                                             start=True, stop=False)
                            nc.tensor.matmul(hp[:], lhsT=z1s[1][:, sl],
                                             rhs=W_rT_sb[r, 1][:],
                                             start=False, stop=True)
                            # bias (free-dim) + relu: DVE add then ACT relu+cast
                            nc.vector.tensor_add(hp[:], hp[:], b_r_rep_sb[r][:])
                            nc.scalar.activation(hs[:, c4, :], hp[:], Relu,
                                                 bias=0.0, scale=1.0)
                        row0 = r * BUCKETS[q] + ntl * 512
                        w = nc.sync.dma_start(
                            H_d[q][row0:row0 + 512, :].rearrange(
                                "(c p) d -> p c d", p=128),
                            hs[:])
                        h_writes[q].append(w.ins)

            # ---- phase B: quarter-major gathers + segment-sum ----
            for q in range(NQ):
                for b in range(NBLK):
                    tb = tiles_pb[b][q]
                    if tb == 0:
                        continue
                    ofs = grp_ofs[b, q]
                    msup = wp.tile([128, tb, D], h_dt, tag="msup", bufs=3)
                    g_inst = nc.gpsimd.dma_gather(
                        out_ap=msup[:], in_ap=H_d[q][:],
                        idxs_ap=idxs_sb[:, ofs * 8:(ofs + tb) * 8],
                        num_idxs=tb * 128, num_idxs_reg=tb * 128,
                        elem_size=D, single_packet=False)
                    # dma_gather's DRAM read is not tracked by tile deps;
                    # order it after this quarter's H writes.
                    for w in h_writes[q]:
                        add_dep_helper(g_inst.ins, w,
                                       reason="gather waits on Htab writes")
                    selsup = wp.tile([128, tb, 128], h_dt, tag="selsup", bufs=3)
                    for t in range(tb):
                        nc.vector.tensor_tensor(
                            out=selsup[:, t, :],
                            in0=dloc_sb[:, ofs + t:ofs + t + 1].to_broadcast(
                                [128, 128]),
                            in1=iota_sb[:],
                            op=mybir.AluOpType.is_equal)
                    segp = psBC.tile([128, D], f32, tag="seg", space="PSUM", bufs=2)
                    for t in range(tb):
                        nc.tensor.matmul(segp[:], lhsT=selsup[:, t, :],
                                         rhs=msup[:, t, :],
                                         start=(t == 0), stop=(t == tb - 1))
                    if q == 0:
                        nc.vector.tensor_copy(agg_sb[b][:], segp[:])
                    else:
                        nc.vector.tensor_add(agg_sb[b][:], agg_sb[b][:], segp[:])

                    # ---- phase C: after the last quarter's contribution ----
                    if q != NQ - 1:
                        continue
                    nn = min(128, NPC - b * 128)
                    # cb1: [0:256]=i gates, [256:512]=g then o (sequential)
                    # cb2: [0:128]=transpose scratch, [128:256]=x1,
                    #      [256:384]=x2, [384:512]=out halves (sequential)
                    cb1 = psBC.tile([128, 512], f32, tag="cb1", space="PSUM", bufs=1)
                    cb2 = psBC.tile([128, 512], f32, tag="cb2", space="PSUM", bufs=2)
                    aggT = {}
                    for h in range(2):
                        nc.tensor.transpose(cb2[:, 0:128],
                                            agg_sb[b][:, h * 128:(h + 1) * 128],
                                            ident[:])
                        at = wp.tile([128, 128], mm_c, tag=f"aggT{h}")
                        nc.vector.tensor_copy(at[:], cb2[:, 0:128])
                        aggT[h] = at
                    # i gates
                    for h in range(2):
                        nc.tensor.matmul(cb1[:, 0:256], lhsT=aggT[h][:],
                                         rhs=W_ihT_sb[h][:, 0:256],
                                         start=(h == 0), stop=False)
                    nc.tensor.matmul(cb1[:, 0:256], lhsT=ones_c[:],
                                     rhs=b_g_row_sb[:, 0:256],
                                     start=False, stop=True)
                    si = wp.tile([128, 256], f32, tag="si")
                    nc.scalar.activation(si[:], cb1[:, 0:256], Sig, bias=0.0,
                                         scale=1.0)
                    # g gates
                    for h in range(2):
                        nc.tensor.matmul(cb1[:, 256:512], lhsT=aggT[h][:],
                                         rhs=W_ihT_sb[h][:, 256:512],
                                         start=(h == 0), stop=False)
                    nc.tensor.matmul(cb1[:, 256:512], lhsT=ones_c[:],
                                     rhs=b_g_row_sb[:, 256:512],
                                     start=False, stop=True)
                    tg = wp.tile([128, 256], f32, tag="tg")
                    nc.scalar.activation(tg[:], cb1[:, 256:512], Tanh, bias=0.0,
                                         scale=1.0)
                    # o gates (reuse the same psum region)
                    for h in range(2):
                        nc.tensor.matmul(cb1[:, 256:512], lhsT=aggT[h][:],
                                         rhs=W_ihT_sb[h][:, 512:768],
                                         start=(h == 0), stop=False)
                    nc.tensor.matmul(cb1[:, 256:512], lhsT=ones_c[:],
                                     rhs=b_g_row_sb[:, 512:768],
                                     start=False, stop=True)
                    so = wp.tile([128, 256], f32, tag="so")
                    nc.scalar.activation(so[:], cb1[:, 256:512], Sig, bias=0.0,
                                         scale=1.0)
                    cc = wp.tile([128, 256], f32, tag="cc")
                    nc.vector.tensor_mul(cc[:], si[:], tg[:])
                    tcc = wp.tile([128, 256], f32, tag="tcc")
                    nc.scalar.activation(tcc[:], cc[:], Tanh, bias=0.0, scale=1.0)
                    hn = wp.tile([128, 256], f32, tag="hn")
                    nc.vector.tensor_mul(hn[:], so[:], tcc[:])
                    hnT = {}
                    for h in range(2):
                        nc.tensor.transpose(cb2[:, 0:128],
                                            hn[:, h * 128:(h + 1) * 128],
                                            ident[:])
                        ht = wp.tile([128, 128], mm_c, tag=f"hnT{h}")
                        nc.vector.tensor_copy(ht[:], cb2[:, 0:128])
                        hnT[h] = ht
                    # MLP (transposed activation layout: [feature, node])
                    for h in range(2):
                        nc.tensor.matmul(cb2[:, 128:256], lhsT=W1T_sb[h][:],
                                         rhs=hnT[h][:],
                                         start=(h == 0), stop=(h == 1))
                    x1s = wp.tile([128, 128], mm_c, tag="x1s")
                    nc.scalar.activation(x1s[:], cb2[:, 128:256], Relu,
                                         bias=b1_col_sb[:], scale=1.0)
                    nc.tensor.matmul(cb2[:, 256:384], lhsT=W2T_sb[:], rhs=x1s[:],
                                     start=True, stop=True)
                    x2s = wp.tile([128, 128], mm_c, tag="x2s")
                    nc.scalar.activation(x2s[:], cb2[:, 256:384], Relu,
                                         bias=b2_col_sb[:], scale=1.0)
                    for oh in range(2):
                        nc.tensor.matmul(cb2[:, 384:512],
                                         lhsT=W3T_sb[:, oh * 128:(oh + 1) * 128],
                                         rhs=x2s[:], start=True, stop=True)
                        osb = wp.tile([128, 128], f32, tag=f"osb{oh}")
                        nc.vector.tensor_scalar_add(
                            osb[:], cb2[:, 384:512], b3_col_sb[oh][:])
                        nc.sync.dma_start(
                            outT_d[oh * 128:(oh + 1) * 128, b * 128:b * 128 + nn],
                            osb[:, 0:nn])

    nc.compile()
    return nc


_CACHE = {}


def _get_nc(tiles_key, dtcfg):
    key = (tiles_key, dtcfg)
    if key not in _CACHE:
        _CACHE[key] = _build([list(p) for p in tiles_key], dtcfg)
    return _CACHE[key]


# ----------------------------------------------------------------------------
# public entry
# ----------------------------------------------------------------------------

def kernel(dtcfg="bf16", **inputs) -> np.ndarray:
    src = np.asarray(inputs["src"], dtype=np.int32)
    dst = np.asarray(inputs["dst"], dtype=np.int32)
    rel = np.asarray(inputs["rel"], dtype=np.int32)

    mm_a, h_dt, mm_c = DT_CFG[dtcfg]
    com = _prep_weights(inputs, _np_dt[mm_a], _np_dt[mm_c])
    tiles_pb, idxs16_all, dloc_all = _prep_edges(src, dst, rel)

    nc = _get_nc(tuple(tuple(p) for p in tiles_pb), dtcfg)

    in_maps = []
    for c in range(NCORES):
        m = dict(com)
        m["idxs16"] = np.ascontiguousarray(idxs16_all[c])
        m["dloc"] = np.ascontiguousarray(dloc_all[c])
        in_maps.append(m)

    res = bass_utils.run_bass_kernel_spmd(nc, in_maps, core_ids=list(range(NCORES)))

    out = np.empty((N_NODES, D_OUT), dtype=np.float32)
    for c in range(NCORES):
        out[c * NPC:(c + 1) * NPC, :] = res.results[c]["outT"].T
    return out
